# revision 28
# baseline (speedup 1.0000x reference)
"""Trainium2 Bass kernel for BlittingStrokeModel (AA polyline rasterization).

Reference semantics: per batch item, 16 AA segments rasterized onto a zero
canvas via point-to-segment distance: cov = clip(lw + 0.5 - dist, 0, 1),
max over segments, broadcast to 3 channels.

Device formulation (distances scaled by 1/16 so fp16 tiles hold d^2):
    Pp  = perpendicular line distance  = (dy*x - dx*y + cP) * s / 16
    u   = along-axis coordinate        = (dx*x + dy*y - c0) * s      (px)
    E   = cap excess = relu(sigma*u - [L if sigma=+1 else 0]) / 16
    d^2 = Pp^2 + E^2 ;  M = min over segments ;  cov = clip(thr - 16*sqrt(M))

Work unit: one (image, row, column-quarter) "row-job" per segment whose
capsule (radius thr+MARG) meets that 128px quarter of that row.  All
per-segment geometry enters through PER-PARTITION scalars, so any 128
row-units pack into one [128,128] op regardless of which image/row they
come from.  Rows are sorted by job count and packed into NCOMP composite
canvases of 128 rows; program structure (ops per composite) is the
per-composite max job count — identical across cores (SPMD), with
neutral padding coefficients for idle partitions.

Per composite the min-chain runs as interleaved custom-DVE ops
(min((Idx*C0+C1)^2 + Src0^2, Src1) for cap slots with Src0 = an E tile
built by one ACT Relu; min((Src0*C0+C1)^2, Src1) with Src0 = xt for line
slots).  A balanced subset of line slots instead uses ACT Square + a
native fp16 tensor_tensor min (2x DVE packing) to equalize V and ACT
load.  Two-sided caps (both endpoint regions in range in one quarter,
rare) use an ACT Abs + tensor_scalar E build.  Finalize is batched on a
single [128, NCOMP*128] fp16 tile: merge, sqrt, clip, one DMA out.

Only a 1-channel fp16 stroke canvas leaves the device (the output is
channel-replicated and images never affect it); the host scatters rows,
casts to fp32, and broadcasts channels.
"""

import numpy as np
from contextlib import ExitStack

B, C, H, W = 8, 3, 512, 512
K = 17
NSEG = K - 1
P = 128
QW = 128          # column-quarter width
NQ = W // QW      # 4
SC = 1.0 / 16.0   # distance scale for fp16 range
MARG = 0.75
PADB = 200.0      # padding bias -> d^2 = 40000 (< fp16 max)
NCORE = 8

_state = {}


# --------------------------------------------------------------------------
# custom DVE ops
# --------------------------------------------------------------------------

def _register_dve_op(name, spec):
    import concourse.dve_ops as dve_ops
    from concourse.dve_ops import DveOp, OPS, _SUB_OPCODE_FOR_NAME, _CUSTOM_DVE_ROW_BASE
    from concourse.dve_spec import lower, _has_src1
    from concourse.dve_uop import DveOpSpec
    from concourse.dve_table_gen import dve_ver_for

    if name in _SUB_OPCODE_FOR_NAME:
        return next(o for o in OPS if o.name == name)
    row = _CUSTOM_DVE_ROW_BASE + len(OPS)
    assert row < 0x20
    _SUB_OPCODE_FOR_NAME[name] = row
    ver = dve_ver_for("TRN2")
    tmp = DveOpSpec(
        name=name, opcode=row, uops=lower(spec, ver=ver), rd1_en=_has_src1(spec)
    )
    op = DveOp(name, spec, subdim=False, uops_sha={ver: tmp.sha(ver)})
    OPS.append(op)
    dve_ops.CUSTOM_DVE_SPECS[name] = spec
    return op


def _get_dve_ops():
    if "ops" in _state:
        return _state["ops"]
    from concourse.dve_spec import Spec, Src0, Src1, C0, C1, sq, minn, maxx, Idx, Zero, One

    def _idx(in0):
        return np.arange(in0.shape[-1], dtype=np.float32)[None, :]

    d2min = _register_dve_op(
        "STROKE_D2MIN_ANT",
        Spec(
            body=minn(sq(Idx * C0 + C1) + sq(Src0), Src1),
            reference=lambda in0, in1, s0, s1, imm2: np.minimum(
                (_idx(in0) * s0 + s1) ** 2 + in0.astype(np.float32) ** 2, in1
            ).astype(np.float32),
        ),
    )
    d2first = _register_dve_op(
        "STROKE_D2_ANT",
        Spec(
            body=sq(Idx * C0 + C1) + sq(Src0),
            reference=lambda in0, in1, s0, s1, imm2: (
                (_idx(in0) * s0 + s1) ** 2 + in0.astype(np.float32) ** 2
            ).astype(np.float32),
        ),
    )
    lmin = _register_dve_op(
        "STROKE_LD2MIN_ANT",
        Spec(
            body=minn(sq(Src0 * C0 + C1), Src1),
            reference=lambda in0, in1, s0, s1, imm2: np.minimum(
                (in0.astype(np.float32) * s0 + s1) ** 2, in1
            ).astype(np.float32),
        ),
    )
    lfirst = _register_dve_op(
        "STROKE_LD2_ANT",
        Spec(
            body=sq(Src0 * C0 + C1),
            reference=lambda in0, in1, s0, s1, imm2: (
                (in0.astype(np.float32) * s0 + s1) ** 2
            ).astype(np.float32),
        ),
    )
    clips = _register_dve_op(
        "STROKE_CLIPS_ANT",
        Spec(
            body=minn(maxx(Src0 * C0 + C1, Zero), One),
            reference=lambda in0, in1, s0, s1, imm2: np.minimum(
                np.maximum(in0.astype(np.float32) * s0 + s1, 0.0), 1.0
            ).astype(np.float32),
        ),
    )
    _state["ops"] = (d2min, d2first, lmin, lfirst, clips)
    return _state["ops"]


# --------------------------------------------------------------------------
# host geometry
# --------------------------------------------------------------------------

def _segments(xy):
    p0, p1 = xy[:-1].copy(), xy[1:].copy()
    d = p1 - p0
    degen = (d[:, 0] ** 2 + d[:, 1] ** 2) < 1e-12
    d[degen, 0] = 1e-6
    p1 = p0 + d
    return p0, p1, d


def _row_xinterval(p0, p1, d, ys, R):
    """Per y in ys: x-interval [xlo, xhi] with dist((x,y), seg) <= R."""
    dx, dy = d
    dd = dx * dx + dy * dy
    s = 1.0 / np.sqrt(dd)
    xlo = np.full(len(ys), np.inf)
    xhi = np.full(len(ys), -np.inf)
    for px, py in (p0, p1):
        h2 = R * R - (ys - py) ** 2
        ok = h2 >= 0.0
        r = np.sqrt(np.maximum(h2, 0.0))
        xlo = np.where(ok, np.minimum(xlo, px - r), xlo)
        xhi = np.where(ok, np.maximum(xhi, px + r), xhi)
    cP = dx * p0[1] - dy * p0[0]
    if abs(dy) > 1e-12:
        for sgn in (-1.0, 1.0):
            x = (sgn * R / s + dx * ys - cP) / dy
            t = ((x - p0[0]) * dx + (ys - p0[1]) * dy) / dd
            ok = (t >= 0.0) & (t <= 1.0)
            xlo = np.where(ok, np.minimum(xlo, x), xlo)
            xhi = np.where(ok, np.maximum(xhi, x), xhi)
    return xlo, xhi


def _ray_rows_hit(pa, pb, ys, xlo, xhi, R):
    """Per y: does segment pa->pb come within R of {y} x [xlo, xhi]?"""
    d = pb - pa
    dd = float(d @ d)

    def pt_rowseg(px, py):
        cx = np.clip(px, xlo, xhi)
        return np.hypot(px - cx, py - ys)

    d1 = pt_rowseg(pa[0], pa[1])
    d2 = pt_rowseg(pb[0], pb[1])

    def pt_seg(qx, qy):
        t = np.clip(((qx - pa[0]) * d[0] + (qy - pa[1]) * d[1]) / max(dd, 1e-18), 0.0, 1.0)
        return np.hypot(pa[0] + t * d[0] - qx, pa[1] + t * d[1] - qy)

    d3 = pt_seg(xlo, ys)
    d4 = pt_seg(xhi, ys)
    best = np.minimum(np.minimum(d1, d2), np.minimum(d3, d4))
    if abs(d[1]) > 1e-15:
        t = (ys - pa[1]) / d[1]
        xc = pa[0] + t * d[0]
        cross = (t >= 0.0) & (t <= 1.0) & (xc >= xlo) & (xc <= xhi)
        best = np.where(cross, 0.0, best)
    return best <= R


def _plan(trajectories, line_width):
    thr = float(np.asarray(line_width).item()) + 0.5
    R = thr + MARG
    FAR = 1500.0
    xy_all = np.asarray(trajectories, dtype=np.float64)[:, :, 1:3]
    nb = xy_all.shape[0]

    geo = {}
    rows = {}
    ys_full = np.arange(H, dtype=np.float64)
    for b in range(nb):
        p0a, p1a, da = _segments(xy_all[b])
        gl = []
        for s in range(NSEG):
            p0, p1, d = p0a[s], p1a[s], da[s]
            dx, dy = d
            dd = dx * dx + dy * dy
            sc = 1.0 / np.sqrt(dd)
            gl.append(
                dict(
                    dx=dx, dy=dy, s=sc, L=np.sqrt(dd),
                    cP=dx * p0[1] - dy * p0[0],
                    c0u=dx * p0[0] + dy * p0[1],
                )
            )
            ylo = max(0, int(np.ceil(min(p0[1], p1[1]) - R)))
            yhi = min(H - 1, int(np.floor(max(p0[1], p1[1]) + R)))
            if ylo > yhi:
                continue
            ys = ys_full[ylo : yhi + 1]
            xlo, xhi = _row_xinterval(p0, p1, d, ys, R)
            dirv = d / max(float(np.hypot(*d)), 1e-9)
            for h in range(NQ):
                wlo, whi = h * QW - MARG, h * QW + QW - 1 + MARG
                act = (xhi >= wlo) & (xlo <= whi) & (xlo <= xhi)
                if not act.any():
                    continue
                exlo = np.maximum(xlo, wlo)
                exhi = np.minimum(xhi, whi)
                c0 = _ray_rows_hit(p0, p0 - dirv * FAR, ys, exlo, exhi, R) & act
                c1 = _ray_rows_hit(p1, p1 + dirv * FAR, ys, exlo, exhi, R) & act
                for i in np.nonzero(act)[0]:
                    y = ylo + i
                    kind = 2 if (c0[i] and c1[i]) else (-1 if c0[i] else (1 if c1[i] else 0))
                    rows.setdefault((b, y, h), []).append((s, kind))
        geo[b] = gl

    # sort rows by (njobs, ncaps, nabs) desc; blocks of 1024 -> composites
    def rkey(item):
        jl = item[1]
        return (len(jl), sum(1 for _, k in jl if k != 0), sum(1 for _, k in jl if k == 2))

    rlist = sorted(rows.items(), key=rkey, reverse=True)
    ncomp = max(1, (len(rlist) + NCORE * P - 1) // (NCORE * P))
    nj, ncap, nabs = [], [], []
    assign = [[[None] * P for _ in range(ncomp)] for _ in range(NCORE)]
    for c in range(ncomp):
        blk = rlist[c * NCORE * P : (c + 1) * NCORE * P]
        nj.append(max(rkey(it)[0] for it in blk))
        ncap.append(max(rkey(it)[1] for it in blk))
        nabs.append(max(rkey(it)[2] for it in blk))
        for i, (key, jl) in enumerate(blk):
            core, part = i % NCORE, i // NCORE
            lines = [s for s, k in jl if k == 0]
            caps1 = [(s, k) for s, k in jl if k in (-1, 1)]
            caps2 = [(s, 2) for s, k in jl if k == 2]
            assign[core][c][part] = (key[0], key[1], key[2], lines, caps1, caps2)
    struct = (tuple(nj), tuple(ncap), tuple(nabs))
    return struct, assign, thr, geo


# --------------------------------------------------------------------------
# deterministic op-mode derivation (shared by build and prep)
# --------------------------------------------------------------------------

def _derive_modes(struct):
    """Returns per-composite: number of line slots flipped to the native
    ACT-Square + fp16 TT-min path (flipped = the FIRST k line slots).
    Cost constants are measured on HW (incl. per-op semaphore overhead)."""
    nj, ncap, nabs = struct
    ncomp = len(nj)
    CUST, TTF, EACT, TSA, MERGE = 535.0, 346.0, 520.0, 250.0, 346.0
    BIGN = ncomp * QW
    # V: customs + abs-TS + merges + 2 batched clips; ACT: E relus + 2 sqrts
    v = sum(nj) * CUST + sum(nabs) * TSA + 2 * ((BIGN / 2 + 256) / 0.96 + 130)
    a = sum(ncap) * EACT + 2 * ((BIGN / 2 + 352) / 1.2 + 123) + 2566
    merges = sum(1 for c in range(ncomp) if nj[c] >= 4)
    v += merges * MERGE
    kmax = [nj[c] - ncap[c] for c in range(ncomp)]
    k = [0] * ncomp
    order = []
    idx = [0] * ncomp
    while len(order) < sum(kmax):
        for c in range(ncomp):
            if idx[c] < kmax[c]:
                order.append(c)
                idx[c] += 1
    for c in order:
        if v <= a + CUST:
            break
        v += -CUST + TTF
        a += EACT
        k[c] += 1
    return k


# --------------------------------------------------------------------------
# program build (per structure, cached)
# --------------------------------------------------------------------------

def _tab_layout(struct, knat):
    """Hot table (tb): line/cap Pp coefficients, needed by the first V ops.
    Cold table (tc): E-builder + Square + clip scalars, needed later."""
    nj, ncap, nabs = struct
    G, NC, NA, NN = sum(nj), sum(ncap), sum(nabs), sum(knat)
    o = {}
    o["cA"] = 0
    o["cB"] = G
    o["TB"] = 2 * G
    o["rS"] = 0
    o["rB"] = NC
    o["hT"] = 2 * NC
    o["sA"] = 2 * NC + NA
    o["sB"] = o["sA"] + NN
    o["fin"] = o["sB"] + NN
    o["TC"] = o["fin"] + 2
    return o


def _build_program(struct):
    import concourse.tile as tile
    from concourse import bacc, mybir

    dt = mybir.dt
    op = mybir.AluOpType
    af = mybir.ActivationFunctionType
    d2min_op, d2first_op, lmin_op, lfirst_op, clips_op = _get_dve_ops()
    nj, ncap, nabs = struct
    ncomp = len(nj)
    knat = _derive_modes(struct)
    goff = [sum(nj[:c]) for c in range(ncomp)]
    coff = [sum(ncap[:c]) for c in range(ncomp)]
    aoff = [sum(nabs[:c]) for c in range(ncomp)]
    noff = [sum(knat[:c]) for c in range(ncomp)]
    NC, NA, NN = sum(ncap), sum(nabs), sum(knat)
    BIGN = ncomp * QW
    L = _tab_layout(struct, knat)

    nc = bacc.Bacc("TRN2", target_bir_lowering=False, debug=False)
    tb_d = nc.dram_tensor("tb", [P, L["TB"]], dt.float32, kind="ExternalInput").ap()
    tc_d = nc.dram_tensor("tc", [P, L["TC"]], dt.float32, kind="ExternalInput").ap()
    out_d = nc.dram_tensor("out", [P, BIGN], dt.float16, kind="ExternalOutput").ap()

    with tile.TileContext(nc) as tc, ExitStack() as ctx:
        const = ctx.enter_context(tc.tile_pool(name="const", bufs=1))
        tb = const.tile_from(tb_d)
        tcold = const.tile_from(tc_d)
        xt = const.tile([P, QW], dt.float16, name="xt")
        nc.gpsimd.iota(
            xt[:], [[1, QW]], base=0, channel_multiplier=0,
            allow_small_or_imprecise_dtypes=True,
        )

        def T(key, i):
            offi = L[key] + i
            src = tb if key in ("cA", "cB") else tcold
            return src[:, offi : offi + 1]

        big = ctx.enter_context(tc.tile_pool(name="big", bufs=1))
        BQ = big.tile([P, BIGN], dt.float16, name="BQ")
        BD = big.tile([P, BIGN], dt.float16, name="BD")
        BO = big.tile([P, BIGN], dt.float16, name="BO")

        epool = ctx.enter_context(tc.tile_pool(name="e", bufs=max(1, NC + NA)))
        npool = ctx.enter_context(tc.tile_pool(name="n", bufs=max(1, NN)))
        mpool = ctx.enter_context(tc.tile_pool(name="m", bufs=6 * ncomp + 8))
        wpool = ctx.enter_context(tc.tile_pool(name="w", bufs=4))

        # ACT warmup: Sqrt first -> its table set (which also carries the
        # relu/abs/square fillers) is the only ACT_TABLE_LOAD in the program
        wu = wpool.tile([P, 8], dt.float32, name="wu")
        nc.vector.memset(wu[:], 0.0)
        wu2 = wpool.tile([P, 8], dt.float32, name="wu2")
        for fn in (af.Sqrt, af.Square, af.Relu, af.Abs):
            nc.scalar.activation(wu2[:], wu[:], fn)
        # chain schedules: line customs, then caps, then native folds
        scheds, nchains = [], []
        for c in range(ncomp):
            steps = (
                [("L", j) for j in range(knat[c], nj[c] - ncap[c])]
                + [("C", j) for j in range(nj[c] - ncap[c], nj[c])]
                + [("F", jn) for jn in range(knat[c])]
            )
            scheds.append(steps)
            nchains.append(4 if nj[c] >= 6 else 2 if nj[c] >= 4 else 1)

        # ACT producers: E tiles first (deep composites first), then Squares
        ntiles = {c: {} for c in range(ncomp)}
        etiles = {c: {} for c in range(ncomp)}
        for c in range(ncomp):
            for jc in range(ncap[c]):
                j = nj[c] - ncap[c] + jc
                gc = coff[c] + jc
                E = epool.tile([P, QW], dt.float16, tag="E", name=f"E{c}_{jc}")
                if jc >= ncap[c] - nabs[c]:
                    ja = aoff[c] + (jc - (ncap[c] - nabs[c]))
                    At = epool.tile([P, QW], dt.float32, tag="A32", name=f"At{c}_{jc}")
                    nc.scalar.activation(
                        At[:], xt[:], af.Abs, bias=T("rB", gc), scale=T("rS", gc)
                    )
                    nc.vector.tensor_scalar(
                        E[:], At[:], T("hT", ja), 0.0, op0=op.subtract, op1=op.max
                    )
                else:
                    nc.scalar.activation(
                        E[:], xt[:], af.Relu, bias=T("rB", gc), scale=T("rS", gc)
                    )
                etiles[c][j] = E
        for c in range(ncomp):
            for jn in range(knat[c]):
                gn = noff[c] + jn
                # a single-step composite writes its Square straight to BQ
                if len(scheds[c]) == 1 and nchains[c] == 1 and scheds[c][0][0] == "F":
                    ntiles[c][jn] = None
                    nc.scalar.activation(
                        BQ[:, c * QW : (c + 1) * QW], xt[:], af.Square,
                        bias=T("sB", gn), scale=T("sA", gn),
                    )
                    continue
                P2 = npool.tile([P, QW], dt.float16, tag="P2", name=f"P2_{c}_{jn}")
                nc.scalar.activation(
                    P2[:], xt[:], af.Square, bias=T("sB", gn), scale=T("sA", gn)
                )
                ntiles[c][jn] = P2

        chains = [[None] * 4 for _ in range(ncomp)]
        emitted = [0] * ncomp

        def emit_step(c, i):
            kind, j = scheds[c][i]
            ci = i % nchains[c]
            prev = chains[c][ci]  # AP or None
            last = i == len(scheds[c]) - 1 and nchains[c] == 1
            sl = BQ[:, c * QW : (c + 1) * QW]
            if kind in ("L", "C"):
                g = goff[c] + j
                out = sl if last else mpool.tile([P, QW], dt.float16, tag="M", name=f"M{c}_{i}")[:]
                iscap = kind == "C"
                src0 = etiles[c][j][:] if iscap else xt[:]
                if prev is None:
                    nc.vector._custom_dve(
                        d2first_op if iscap else lfirst_op,
                        out=out, in0=src0, s0=T("cA", g), s1=T("cB", g),
                    )
                else:
                    nc.vector._custom_dve(
                        d2min_op if iscap else lmin_op,
                        out=out, in0=src0, in1=prev,
                        s0=T("cA", g), s1=T("cB", g),
                    )
                chains[c][ci] = out
            else:
                P2 = ntiles[c][j]
                if P2 is None:  # already written straight to BQ
                    chains[c][ci] = sl
                    return
                if prev is None:
                    chains[c][ci] = P2[:]
                else:
                    out = sl if last else mpool.tile([P, QW], dt.float16, tag="M", name=f"M{c}_{i}")[:]
                    nc.vector.tensor_tensor(out, prev, P2[:], op=op.min)
                    chains[c][ci] = out

        # V emission: all line-customs, then caps, then folds (round-robin)
        for phase in ("L", "C", "F"):
            progressed = True
            while progressed:
                progressed = False
                for c in range(ncomp):
                    i = emitted[c]
                    if i < len(scheds[c]) and scheds[c][i][0] == phase:
                        emit_step(c, i)
                        emitted[c] += 1
                        progressed = True

        # all merges first (lightest composites first), then 4 finalize
        # groups so the heavy group's serial tail isn't queued behind
        # other groups' clips
        for c in range(ncomp - 1, -1, -1):
            live = [x for x in chains[c] if x is not None]
            sl = BQ[:, c * QW : (c + 1) * QW]
            if len(live) == 2:
                nc.vector.tensor_tensor(sl, live[0], live[1], op=op.min)
            elif len(live) >= 3:
                m1 = mpool.tile([P, QW], dt.float16, tag="M", name=f"mg{c}a")
                nc.vector.tensor_tensor(m1[:], live[0], live[1], op=op.min)
                if len(live) == 4:
                    m2 = mpool.tile([P, QW], dt.float16, tag="M", name=f"mg{c}b")
                    nc.vector.tensor_tensor(m2[:], live[2], live[3], op=op.min)
                    nc.vector.tensor_tensor(sl, m1[:], m2[:], op=op.min)
                else:
                    nc.vector.tensor_tensor(sl, m1[:], live[2], op=op.min)
        ng = min(4, ncomp)
        bounds = [round(i * ncomp / ng) for i in range(ng + 1)]
        groups = [list(range(bounds[i], bounds[i + 1])) for i in range(ng)][::-1]
        BR = big.tile([P, BIGN], dt.float16, name="BR")
        for gi, comps in enumerate(groups):
            lo = min(comps) * QW
            hhi = (max(comps) + 1) * QW
            nc.scalar.activation(BD[:, lo:hhi], BQ[:, lo:hhi], af.Sqrt)
            if gi < len(groups) - 1:
                # early groups: clip on ACT (relu) + cheap V min — V is the
                # busy engine during these; last group keeps the 1-op custom
                nc.scalar.activation(
                    BR[:, lo:hhi], BD[:, lo:hhi], af.Relu,
                    bias=T("fin", 1), scale=T("fin", 0),
                )
                nc.vector.tensor_scalar(
                    BO[:, lo:hhi], BR[:, lo:hhi], 1.0, 0.0, op0=op.min, op1=op.add
                )
            else:
                nc.vector._custom_dve(
                    clips_op, out=BO[:, lo:hhi], in0=BD[:, lo:hhi],
                    s0=T("fin", 0), s1=T("fin", 1),
                )
            nc.sync.dma_start(out_d[:, lo:hhi], BO[:, lo:hhi])

    nc.compile()
    return nc


# --------------------------------------------------------------------------
# host coefficient tables
# --------------------------------------------------------------------------

def _prep_inputs(trajectories, struct, assign, thr, geo):
    nj, ncap, nabs = struct
    ncomp = len(nj)
    knat = _derive_modes(struct)
    G = sum(nj)
    NC = sum(ncap)
    NA = sum(nabs)
    NN = sum(knat)
    goff = [sum(nj[:c]) for c in range(ncomp)]
    coff = [sum(ncap[:c]) for c in range(ncomp)]
    aoff = [sum(nabs[:c]) for c in range(ncomp)]
    noff = [sum(knat[:c]) for c in range(ncomp)]
    L = _tab_layout(struct, knat)

    xt = np.broadcast_to(np.arange(QW, dtype=np.float32), (P, QW)).astype(np.float16)
    in_maps = []
    for core in range(NCORE):
        cA = np.zeros((P, G))
        cB = np.full((P, G), PADB)
        rS = np.zeros((P, max(1, NC)))
        rB = np.full((P, max(1, NC)), -1.0)
        hT = np.ones((P, max(1, NA)))
        sA = np.zeros((P, max(1, NN)))
        sB = np.full((P, max(1, NN)), PADB)
        for c in range(ncomp):
            nline_slots = nj[c] - ncap[c]
            for p in range(P):
                ent = assign[core][c][p]
                if ent is None:
                    continue
                b, y, h = ent[0], ent[1], ent[2]
                lines, caps1, caps2 = ent[3], ent[4], ent[5]
                xoff = float(h * QW)
                gl = geo[b]

                def pp_coef(s):
                    gg = gl[s]
                    a = gg["dy"] * gg["s"] * SC
                    bb = (gg["dy"] * xoff - gg["dx"] * y + gg["cP"]) * gg["s"] * SC
                    return a, bb

                # line jobs: first knat slots are native, rest custom
                for li, s in enumerate(lines):
                    a, bb = pp_coef(s)
                    if li < knat[c]:
                        sA[p, noff[c] + li] = a
                        sB[p, noff[c] + li] = bb
                    else:
                        g = goff[c] + li
                        cA[p, g] = a
                        cB[p, g] = bb
                # cap jobs fill from the END; two-sided first (into abs slots)
                for ci_, (s, kind) in enumerate(caps2 + caps1):
                    j = nj[c] - 1 - ci_
                    jc = j - nline_slots
                    g = goff[c] + j
                    gc = coff[c] + jc
                    a, bb = pp_coef(s)
                    cA[p, g] = a
                    cB[p, g] = bb
                    gg = gl[s]
                    ub = (gg["dx"] * xoff + gg["dy"] * y - gg["c0u"]) * gg["s"]
                    if jc >= ncap[c] - nabs[c]:
                        # abs flavor: At = |u - L/2| * SC ; E = max(At - h', 0)
                        rS[p, gc] = gg["dx"] * gg["s"] * SC
                        rB[p, gc] = (ub - gg["L"] / 2.0) * SC
                        hT[p, aoff[c] + (jc - (ncap[c] - nabs[c]))] = gg["L"] / 2.0 * SC
                    elif kind == 1:
                        rS[p, gc] = gg["dx"] * gg["s"] * SC
                        rB[p, gc] = (ub - gg["L"]) * SC
                    else:
                        rS[p, gc] = -gg["dx"] * gg["s"] * SC
                        rB[p, gc] = -ub * SC
        tb = np.zeros((P, L["TB"]))
        tb[:, L["cA"] : L["cA"] + G] = cA
        tb[:, L["cB"] : L["cB"] + G] = cB
        tc = np.zeros((P, L["TC"]))
        if NC:
            tc[:, L["rS"] : L["rS"] + NC] = rS
            tc[:, L["rB"] : L["rB"] + NC] = rB
        if NA:
            tc[:, L["hT"] : L["hT"] + NA] = hT
        if NN:
            tc[:, L["sA"] : L["sA"] + NN] = sA
            tc[:, L["sB"] : L["sB"] + NN] = sB
        tc[:, L["fin"]] = -1.0 / SC
        tc[:, L["fin"] + 1] = thr
        in_maps.append({"tb": tb.astype(np.float32), "tc": tc.astype(np.float32)})
    return in_maps


def kernel(**inputs):
    from concourse.bass_utils import run_bass_kernel_spmd

    images = np.asarray(inputs["images"])
    trajectories = np.asarray(inputs["trajectories"])
    line_width = inputs["line_width"]
    assert images.shape == (B, C, H, W), images.shape

    struct, assign, thr, geo = _plan(trajectories, line_width)
    progs = _state.setdefault("progs", {})
    if struct not in progs:
        progs[struct] = _build_program(struct)
    nc = progs[struct]

    in_maps = _prep_inputs(trajectories, struct, assign, thr, geo)
    res = run_bass_kernel_spmd(nc, in_maps, list(range(NCORE))).results
    ncomp = len(struct[0])
    out = np.zeros((B, H, W), np.float32)
    for core in range(NCORE):
        blk = res[core]["out"].astype(np.float32)  # [P, ncomp*QW]
        for c in range(ncomp):
            for p in range(P):
                ent = assign[core][c][p]
                if ent is None:
                    continue
                b, y, h = ent[0], ent[1], ent[2]
                out[b, y, h * QW : (h + 1) * QW] = blk[p, c * QW : (c + 1) * QW]
    full = np.broadcast_to(out[:, None, :, :], (B, C, H, W)).copy()
    return full


if __name__ == "__main__":
    rng = np.random.default_rng(0)
    ins = {
        "images": rng.standard_normal((B, C, H, W)).astype(np.float32),
        "trajectories": np.concatenate(
            [
                np.broadcast_to(np.linspace(0, 1, K, dtype=np.float32), (B, K))[..., None],
                rng.uniform(0, W - 1, (B, K, 2)).astype(np.float32),
                np.ones((B, K, 1), np.float32),
            ],
            axis=-1,
        ),
        "line_width": 3,
    }
    out = kernel(**ins)
    print(out.shape, out.dtype, out.min(), out.max())


# revision 30
# speedup vs baseline: 1.1891x; 1.1891x over previous
"""Trainium2 Bass kernel for BlittingStrokeModel (AA polyline rasterization).

Reference semantics: per batch item, 16 AA segments rasterized onto a zero
canvas via point-to-segment distance: cov = clip(lw + 0.5 - dist, 0, 1),
max over segments, broadcast to 3 channels.

Device formulation (distances scaled by 1/16 so fp16 tiles hold d^2):
    Pp  = perpendicular line distance  = (dy*x - dx*y + cP) * s / 16
    u   = along-axis coordinate        = (dx*x + dy*y - c0) * s      (px)
    E   = cap excess = relu(sigma*u - [L if sigma=+1 else 0]) / 16
    d^2 = Pp^2 + E^2 ;  M = min over segments ;  cov = clip(thr - 16*sqrt(M))

Work unit: one (image, row, column-quarter) "row-job" per segment whose
capsule (radius thr+MARG) meets that 128px quarter of that row.  All
per-segment geometry enters through PER-PARTITION scalars, so any 128
row-units pack into one [128,128] op regardless of which image/row they
come from.  Rows are sorted by job count and packed into NCOMP composite
canvases of 128 rows; program structure (ops per composite) is the
per-composite max job count — identical across cores (SPMD), with
neutral padding coefficients for idle partitions.

Per composite the min-chain runs as interleaved custom-DVE ops
(min((Idx*C0+C1)^2 + Src0^2, Src1) for cap slots with Src0 = an E tile
built by one ACT Relu; min((Src0*C0+C1)^2, Src1) with Src0 = xt for line
slots).  A balanced subset of line slots instead uses ACT Square + a
native fp16 tensor_tensor min (2x DVE packing) to equalize V and ACT
load.  Two-sided caps (both endpoint regions in range in one quarter,
rare) use an ACT Abs + tensor_scalar E build.  Finalize is batched on a
single [128, NCOMP*128] fp16 tile: merge, sqrt, clip, one DMA out.

Only a 1-channel fp16 stroke canvas leaves the device (the output is
channel-replicated and images never affect it); the host scatters rows,
casts to fp32, and broadcasts channels.
"""

import numpy as np
from contextlib import ExitStack

B, C, H, W = 8, 3, 512, 512
K = 17
NSEG = K - 1
P = 128
QW = 128          # column-quarter width
NQ = W // QW      # 4
SC = 1.0 / 16.0   # distance scale for fp16 range
MARG = 0.75
PADB = 200.0      # padding bias -> d^2 = 40000 (< fp16 max)
NCORE = 8

_state = {}


# --------------------------------------------------------------------------
# custom DVE ops
# --------------------------------------------------------------------------

def _register_dve_op(name, spec):
    import concourse.dve_ops as dve_ops
    from concourse.dve_ops import DveOp, OPS, _SUB_OPCODE_FOR_NAME, _CUSTOM_DVE_ROW_BASE
    from concourse.dve_spec import lower, _has_src1
    from concourse.dve_uop import DveOpSpec
    from concourse.dve_table_gen import dve_ver_for

    if name in _SUB_OPCODE_FOR_NAME:
        return next(o for o in OPS if o.name == name)
    row = _CUSTOM_DVE_ROW_BASE + len(OPS)
    assert row < 0x20
    _SUB_OPCODE_FOR_NAME[name] = row
    ver = dve_ver_for("TRN2")
    tmp = DveOpSpec(
        name=name, opcode=row, uops=lower(spec, ver=ver), rd1_en=_has_src1(spec)
    )
    op = DveOp(name, spec, subdim=False, uops_sha={ver: tmp.sha(ver)})
    OPS.append(op)
    dve_ops.CUSTOM_DVE_SPECS[name] = spec
    return op


def _get_dve_ops():
    if "ops" in _state:
        return _state["ops"]
    from concourse.dve_spec import Spec, Src0, Src1, C0, C1, sq, minn, maxx, Idx, Zero, One

    def _idx(in0):
        return np.arange(in0.shape[-1], dtype=np.float32)[None, :]

    d2min = _register_dve_op(
        "STROKE_D2MIN_ANT",
        Spec(
            body=minn(sq(Idx * C0 + C1) + sq(Src0), Src1),
            reference=lambda in0, in1, s0, s1, imm2: np.minimum(
                (_idx(in0) * s0 + s1) ** 2 + in0.astype(np.float32) ** 2, in1
            ).astype(np.float32),
        ),
    )
    d2first = _register_dve_op(
        "STROKE_D2_ANT",
        Spec(
            body=sq(Idx * C0 + C1) + sq(Src0),
            reference=lambda in0, in1, s0, s1, imm2: (
                (_idx(in0) * s0 + s1) ** 2 + in0.astype(np.float32) ** 2
            ).astype(np.float32),
        ),
    )
    lmin = _register_dve_op(
        "STROKE_LD2MIN_ANT",
        Spec(
            body=minn(sq(Src0 * C0 + C1), Src1),
            reference=lambda in0, in1, s0, s1, imm2: np.minimum(
                (in0.astype(np.float32) * s0 + s1) ** 2, in1
            ).astype(np.float32),
        ),
    )
    lfirst = _register_dve_op(
        "STROKE_LD2_ANT",
        Spec(
            body=sq(Src0 * C0 + C1),
            reference=lambda in0, in1, s0, s1, imm2: (
                (in0.astype(np.float32) * s0 + s1) ** 2
            ).astype(np.float32),
        ),
    )
    clips = _register_dve_op(
        "STROKE_CLIPS_ANT",
        Spec(
            body=minn(maxx(Src0 * C0 + C1, Zero), One),
            reference=lambda in0, in1, s0, s1, imm2: np.minimum(
                np.maximum(in0.astype(np.float32) * s0 + s1, 0.0), 1.0
            ).astype(np.float32),
        ),
    )
    _state["ops"] = (d2min, d2first, lmin, lfirst, clips)
    return _state["ops"]


# --------------------------------------------------------------------------
# host geometry
# --------------------------------------------------------------------------

def _segments(xy):
    p0, p1 = xy[:-1].copy(), xy[1:].copy()
    d = p1 - p0
    degen = (d[:, 0] ** 2 + d[:, 1] ** 2) < 1e-12
    d[degen, 0] = 1e-6
    p1 = p0 + d
    return p0, p1, d


def _row_xinterval(p0, p1, d, ys, R):
    """Per y in ys: x-interval [xlo, xhi] with dist((x,y), seg) <= R."""
    dx, dy = d
    dd = dx * dx + dy * dy
    s = 1.0 / np.sqrt(dd)
    xlo = np.full(len(ys), np.inf)
    xhi = np.full(len(ys), -np.inf)
    for px, py in (p0, p1):
        h2 = R * R - (ys - py) ** 2
        ok = h2 >= 0.0
        r = np.sqrt(np.maximum(h2, 0.0))
        xlo = np.where(ok, np.minimum(xlo, px - r), xlo)
        xhi = np.where(ok, np.maximum(xhi, px + r), xhi)
    cP = dx * p0[1] - dy * p0[0]
    if abs(dy) > 1e-12:
        for sgn in (-1.0, 1.0):
            x = (sgn * R / s + dx * ys - cP) / dy
            t = ((x - p0[0]) * dx + (ys - p0[1]) * dy) / dd
            ok = (t >= 0.0) & (t <= 1.0)
            xlo = np.where(ok, np.minimum(xlo, x), xlo)
            xhi = np.where(ok, np.maximum(xhi, x), xhi)
    return xlo, xhi


def _ray_rows_hit(pa, pb, ys, xlo, xhi, R):
    """Per y: does segment pa->pb come within R of {y} x [xlo, xhi]?"""
    d = pb - pa
    dd = float(d @ d)

    def pt_rowseg(px, py):
        cx = np.clip(px, xlo, xhi)
        return np.hypot(px - cx, py - ys)

    d1 = pt_rowseg(pa[0], pa[1])
    d2 = pt_rowseg(pb[0], pb[1])

    def pt_seg(qx, qy):
        t = np.clip(((qx - pa[0]) * d[0] + (qy - pa[1]) * d[1]) / max(dd, 1e-18), 0.0, 1.0)
        return np.hypot(pa[0] + t * d[0] - qx, pa[1] + t * d[1] - qy)

    d3 = pt_seg(xlo, ys)
    d4 = pt_seg(xhi, ys)
    best = np.minimum(np.minimum(d1, d2), np.minimum(d3, d4))
    if abs(d[1]) > 1e-15:
        t = (ys - pa[1]) / d[1]
        xc = pa[0] + t * d[0]
        cross = (t >= 0.0) & (t <= 1.0) & (xc >= xlo) & (xc <= xhi)
        best = np.where(cross, 0.0, best)
    return best <= R


def _plan(trajectories, line_width):
    thr = float(np.asarray(line_width).item()) + 0.5
    R = thr + MARG
    FAR = 1500.0
    xy_all = np.asarray(trajectories, dtype=np.float64)[:, :, 1:3]
    nb = xy_all.shape[0]

    geo = {}
    rows = {}
    ys_full = np.arange(H, dtype=np.float64)
    for b in range(nb):
        p0a, p1a, da = _segments(xy_all[b])
        gl = []
        for s in range(NSEG):
            p0, p1, d = p0a[s], p1a[s], da[s]
            dx, dy = d
            dd = dx * dx + dy * dy
            sc = 1.0 / np.sqrt(dd)
            gl.append(
                dict(
                    dx=dx, dy=dy, s=sc, L=np.sqrt(dd),
                    cP=dx * p0[1] - dy * p0[0],
                    c0u=dx * p0[0] + dy * p0[1],
                )
            )
            ylo = max(0, int(np.ceil(min(p0[1], p1[1]) - R)))
            yhi = min(H - 1, int(np.floor(max(p0[1], p1[1]) + R)))
            if ylo > yhi:
                continue
            ys = ys_full[ylo : yhi + 1]
            xlo, xhi = _row_xinterval(p0, p1, d, ys, R)
            dirv = d / max(float(np.hypot(*d)), 1e-9)
            for h in range(NQ):
                wlo, whi = h * QW - MARG, h * QW + QW - 1 + MARG
                act = (xhi >= wlo) & (xlo <= whi) & (xlo <= xhi)
                if not act.any():
                    continue
                exlo = np.maximum(xlo, wlo)
                exhi = np.minimum(xhi, whi)
                c0 = _ray_rows_hit(p0, p0 - dirv * FAR, ys, exlo, exhi, R) & act
                c1 = _ray_rows_hit(p1, p1 + dirv * FAR, ys, exlo, exhi, R) & act
                for i in np.nonzero(act)[0]:
                    y = ylo + i
                    kind = 2 if (c0[i] and c1[i]) else (-1 if c0[i] else (1 if c1[i] else 0))
                    rows.setdefault((b, y, h), []).append((s, kind))
        geo[b] = gl

    # sort rows by (njobs, ncaps, nabs) desc; blocks of 1024 -> composites
    def rkey(item):
        jl = item[1]
        return (len(jl), sum(1 for _, k in jl if k != 0), sum(1 for _, k in jl if k == 2))

    rlist = sorted(rows.items(), key=rkey, reverse=True)
    ncomp = max(1, (len(rlist) + NCORE * P - 1) // (NCORE * P))
    nj, ncap, nabs = [], [], []
    assign = [[[None] * P for _ in range(ncomp)] for _ in range(NCORE)]
    for c in range(ncomp):
        blk = rlist[c * NCORE * P : (c + 1) * NCORE * P]
        nj.append(max(rkey(it)[0] for it in blk))
        ncap.append(max(rkey(it)[1] for it in blk))
        nabs.append(max(rkey(it)[2] for it in blk))
        for i, (key, jl) in enumerate(blk):
            core, part = i % NCORE, i // NCORE
            lines = [s for s, k in jl if k == 0]
            caps1 = [(s, k) for s, k in jl if k in (-1, 1)]
            caps2 = [(s, 2) for s, k in jl if k == 2]
            assign[core][c][part] = (key[0], key[1], key[2], lines, caps1, caps2)
    struct = (tuple(nj), tuple(ncap), tuple(nabs))
    return struct, assign, thr, geo


# --------------------------------------------------------------------------
# deterministic op-mode derivation (shared by build and prep)
# --------------------------------------------------------------------------

def _derive_modes(struct):
    """Returns per-composite: number of line slots flipped to the native
    ACT-Square + fp16 TT-min path (flipped = the FIRST k line slots).
    Cost constants are measured on HW (incl. per-op semaphore overhead)."""
    nj, ncap, nabs = struct
    ncomp = len(nj)
    CUST, TTF, EACT, TSA, MERGE = 535.0, 346.0, 650.0, 250.0, 346.0
    BIGN = ncomp * QW
    # V: customs + abs-TS + merges + 2 batched clips; ACT: E relus + 2 sqrts
    v = sum(nj) * CUST + sum(nabs) * TSA + 2 * ((BIGN / 2 + 256) / 0.96 + 130)
    a = sum(ncap) * EACT + 2 * ((BIGN / 2 + 352) / 1.2 + 123) + 2566
    merges = sum(1 for c in range(ncomp) if nj[c] >= 4)
    v += merges * MERGE
    kmax = [nj[c] - ncap[c] for c in range(ncomp)]
    k = [0] * ncomp
    order = []
    idx = [0] * ncomp
    while len(order) < sum(kmax):
        for c in range(ncomp):
            if idx[c] < kmax[c]:
                order.append(c)
                idx[c] += 1
    for c in order:
        if v <= a + CUST:
            break
        v += -CUST + TTF
        a += EACT
        k[c] += 1
    return k


# --------------------------------------------------------------------------
# program build (per structure, cached)
# --------------------------------------------------------------------------

def _tab_layout(struct, knat):
    """Hot table (tb): line/cap Pp coefficients, needed by the first V ops.
    Cold table (tc): E-builder + Square + clip scalars, needed later."""
    nj, ncap, nabs = struct
    G, NC, NA, NN = sum(nj), sum(ncap), sum(nabs), sum(knat)
    o = {}
    o["cA"] = 0
    o["cB"] = G
    o["TB"] = 2 * G
    o["rS"] = 0
    o["rB"] = NC
    o["hT"] = 2 * NC
    o["sA"] = 2 * NC + NA
    o["sB"] = o["sA"] + NN
    o["fin"] = o["sB"] + NN
    o["TC"] = o["fin"] + 2
    return o


def _build_program(struct):
    import concourse.tile as tile
    from concourse import bacc, mybir

    dt = mybir.dt
    op = mybir.AluOpType
    af = mybir.ActivationFunctionType
    d2min_op, d2first_op, lmin_op, lfirst_op, clips_op = _get_dve_ops()
    nj, ncap, nabs = struct
    ncomp = len(nj)
    knat = _derive_modes(struct)
    goff = [sum(nj[:c]) for c in range(ncomp)]
    coff = [sum(ncap[:c]) for c in range(ncomp)]
    aoff = [sum(nabs[:c]) for c in range(ncomp)]
    noff = [sum(knat[:c]) for c in range(ncomp)]
    NC, NA, NN = sum(ncap), sum(nabs), sum(knat)
    BIGN = ncomp * QW
    L = _tab_layout(struct, knat)

    nc = bacc.Bacc("TRN2", target_bir_lowering=False, debug=False)
    tb_d = nc.dram_tensor("tb", [P, L["TB"]], dt.float32, kind="ExternalInput").ap()
    tc_d = nc.dram_tensor("tc", [P, L["TC"]], dt.float32, kind="ExternalInput").ap()
    out_d = nc.dram_tensor("out", [P, BIGN], dt.float16, kind="ExternalOutput").ap()

    with tile.TileContext(nc) as tc, ExitStack() as ctx:
        const = ctx.enter_context(tc.tile_pool(name="const", bufs=1))
        tb = const.tile_from(tb_d)
        tcold = const.tile_from(tc_d)
        xt = const.tile([P, QW], dt.float16, name="xt")
        nc.gpsimd.iota(
            xt[:], [[1, QW]], base=0, channel_multiplier=0,
            allow_small_or_imprecise_dtypes=True,
        )

        def T(key, i):
            offi = L[key] + i
            src = tb if key in ("cA", "cB") else tcold
            return src[:, offi : offi + 1]

        big = ctx.enter_context(tc.tile_pool(name="big", bufs=1))
        BQ = big.tile([P, BIGN], dt.float16, name="BQ")
        BD = big.tile([P, BIGN], dt.float16, name="BD")
        BO = big.tile([P, BIGN], dt.float16, name="BO")

        epool = ctx.enter_context(tc.tile_pool(name="e", bufs=max(1, NC + NA)))
        npool = ctx.enter_context(tc.tile_pool(name="n", bufs=max(1, NN)))
        mpool = ctx.enter_context(tc.tile_pool(name="m", bufs=6 * ncomp + 8))
        wpool = ctx.enter_context(tc.tile_pool(name="w", bufs=4))

        # ACT warmup: Sqrt first -> its table set (which also carries the
        # relu/abs/square fillers) is the only ACT_TABLE_LOAD in the program
        wu = wpool.tile([P, 8], dt.float32, name="wu")
        nc.vector.memset(wu[:], 0.0)
        wu2 = wpool.tile([P, 8], dt.float32, name="wu2")
        for fn in (af.Sqrt, af.Square, af.Relu, af.Abs):
            nc.scalar.activation(wu2[:], wu[:], fn)
        # chain schedules: line customs, then caps, then native folds
        scheds, nchains = [], []
        for c in range(ncomp):
            steps = (
                [("L", j) for j in range(knat[c], nj[c] - ncap[c])]
                + [("C", j) for j in range(nj[c] - ncap[c], nj[c])]
                + [("F", jn) for jn in range(knat[c])]
            )
            scheds.append(steps)
            nchains.append(4 if nj[c] >= 6 else 2 if nj[c] >= 4 else 1)

        # ACT producers: E tiles first (deep composites first), then Squares
        ntiles = {c: {} for c in range(ncomp)}
        etiles = {c: {} for c in range(ncomp)}
        for c in range(ncomp):
            for jc in range(ncap[c]):
                j = nj[c] - ncap[c] + jc
                gc = coff[c] + jc
                E = epool.tile([P, QW], dt.float16, tag="E", name=f"E{c}_{jc}")
                if jc >= ncap[c] - nabs[c]:
                    ja = aoff[c] + (jc - (ncap[c] - nabs[c]))
                    At = epool.tile([P, QW], dt.float32, tag="A32", name=f"At{c}_{jc}")
                    nc.scalar.activation(
                        At[:], xt[:], af.Abs, bias=T("rB", gc), scale=T("rS", gc)
                    )
                    nc.vector.tensor_scalar(
                        E[:], At[:], T("hT", ja), 0.0, op0=op.subtract, op1=op.max
                    )
                else:
                    nc.scalar.activation(
                        E[:], xt[:], af.Relu, bias=T("rB", gc), scale=T("rS", gc)
                    )
                etiles[c][j] = E
        for c in range(ncomp):
            for jn in range(knat[c]):
                gn = noff[c] + jn
                # a single-step composite writes its Square straight to BQ
                if len(scheds[c]) == 1 and nchains[c] == 1 and scheds[c][0][0] == "F":
                    ntiles[c][jn] = None
                    nc.scalar.activation(
                        BQ[:, c * QW : (c + 1) * QW], xt[:], af.Square,
                        bias=T("sB", gn), scale=T("sA", gn),
                    )
                    continue
                P2 = npool.tile([P, QW], dt.float16, tag="P2", name=f"P2_{c}_{jn}")
                nc.scalar.activation(
                    P2[:], xt[:], af.Square, bias=T("sB", gn), scale=T("sA", gn)
                )
                ntiles[c][jn] = P2

        chains = [[None] * 4 for _ in range(ncomp)]
        emitted = [0] * ncomp

        def emit_step(c, i):
            kind, j = scheds[c][i]
            ci = i % nchains[c]
            prev = chains[c][ci]  # AP or None
            last = i == len(scheds[c]) - 1 and nchains[c] == 1
            sl = BQ[:, c * QW : (c + 1) * QW]
            if kind in ("L", "C"):
                g = goff[c] + j
                out = sl if last else mpool.tile([P, QW], dt.float16, tag="M", name=f"M{c}_{i}")[:]
                iscap = kind == "C"
                src0 = etiles[c][j][:] if iscap else xt[:]
                if prev is None:
                    nc.vector._custom_dve(
                        d2first_op if iscap else lfirst_op,
                        out=out, in0=src0, s0=T("cA", g), s1=T("cB", g),
                    )
                else:
                    nc.vector._custom_dve(
                        d2min_op if iscap else lmin_op,
                        out=out, in0=src0, in1=prev,
                        s0=T("cA", g), s1=T("cB", g),
                    )
                chains[c][ci] = out
            else:
                P2 = ntiles[c][j]
                if P2 is None:  # already written straight to BQ
                    chains[c][ci] = sl
                    return
                if prev is None:
                    chains[c][ci] = P2[:]
                else:
                    out = sl if last else mpool.tile([P, QW], dt.float16, tag="M", name=f"M{c}_{i}")[:]
                    nc.vector.tensor_tensor(out, prev, P2[:], op=op.min)
                    chains[c][ci] = out

        # V emission: all line-customs, then caps, then folds (round-robin)
        for phase in ("L", "C", "F"):
            progressed = True
            while progressed:
                progressed = False
                for c in range(ncomp):
                    i = emitted[c]
                    if i < len(scheds[c]) and scheds[c][i][0] == phase:
                        emit_step(c, i)
                        emitted[c] += 1
                        progressed = True

        # all merges first (lightest composites first), then 4 finalize
        # groups so the heavy group's serial tail isn't queued behind
        # other groups' clips
        for c in range(ncomp - 1, -1, -1):
            live = [x for x in chains[c] if x is not None]
            sl = BQ[:, c * QW : (c + 1) * QW]
            if len(live) == 2:
                nc.vector.tensor_tensor(sl, live[0], live[1], op=op.min)
            elif len(live) >= 3:
                m1 = mpool.tile([P, QW], dt.float16, tag="M", name=f"mg{c}a")
                nc.vector.tensor_tensor(m1[:], live[0], live[1], op=op.min)
                if len(live) == 4:
                    m2 = mpool.tile([P, QW], dt.float16, tag="M", name=f"mg{c}b")
                    nc.vector.tensor_tensor(m2[:], live[2], live[3], op=op.min)
                    nc.vector.tensor_tensor(sl, m1[:], m2[:], op=op.min)
                else:
                    nc.vector.tensor_tensor(sl, m1[:], live[2], op=op.min)
        ng = min(4, ncomp)
        bounds = [round(i * ncomp / ng) for i in range(ng + 1)]
        groups = [list(range(bounds[i], bounds[i + 1])) for i in range(ng)][::-1]
        for comps in groups:
            lo = min(comps) * QW
            hhi = (max(comps) + 1) * QW
            nc.scalar.activation(BD[:, lo:hhi], BQ[:, lo:hhi], af.Sqrt)
            nc.vector._custom_dve(
                clips_op, out=BO[:, lo:hhi], in0=BD[:, lo:hhi],
                s0=T("fin", 0), s1=T("fin", 1),
            )
            nc.sync.dma_start(out_d[:, lo:hhi], BO[:, lo:hhi])

    nc.compile()
    return nc


# --------------------------------------------------------------------------
# host coefficient tables
# --------------------------------------------------------------------------

def _prep_inputs(trajectories, struct, assign, thr, geo):
    nj, ncap, nabs = struct
    ncomp = len(nj)
    knat = _derive_modes(struct)
    G = sum(nj)
    NC = sum(ncap)
    NA = sum(nabs)
    NN = sum(knat)
    goff = [sum(nj[:c]) for c in range(ncomp)]
    coff = [sum(ncap[:c]) for c in range(ncomp)]
    aoff = [sum(nabs[:c]) for c in range(ncomp)]
    noff = [sum(knat[:c]) for c in range(ncomp)]
    L = _tab_layout(struct, knat)

    xt = np.broadcast_to(np.arange(QW, dtype=np.float32), (P, QW)).astype(np.float16)
    in_maps = []
    for core in range(NCORE):
        cA = np.zeros((P, G))
        cB = np.full((P, G), PADB)
        rS = np.zeros((P, max(1, NC)))
        rB = np.full((P, max(1, NC)), -1.0)
        hT = np.ones((P, max(1, NA)))
        sA = np.zeros((P, max(1, NN)))
        sB = np.full((P, max(1, NN)), PADB)
        for c in range(ncomp):
            nline_slots = nj[c] - ncap[c]
            for p in range(P):
                ent = assign[core][c][p]
                if ent is None:
                    continue
                b, y, h = ent[0], ent[1], ent[2]
                lines, caps1, caps2 = ent[3], ent[4], ent[5]
                xoff = float(h * QW)
                gl = geo[b]

                def pp_coef(s):
                    gg = gl[s]
                    a = gg["dy"] * gg["s"] * SC
                    bb = (gg["dy"] * xoff - gg["dx"] * y + gg["cP"]) * gg["s"] * SC
                    return a, bb

                # line jobs: first knat slots are native, rest custom
                for li, s in enumerate(lines):
                    a, bb = pp_coef(s)
                    if li < knat[c]:
                        sA[p, noff[c] + li] = a
                        sB[p, noff[c] + li] = bb
                    else:
                        g = goff[c] + li
                        cA[p, g] = a
                        cB[p, g] = bb
                # cap jobs fill from the END; two-sided first (into abs slots)
                for ci_, (s, kind) in enumerate(caps2 + caps1):
                    j = nj[c] - 1 - ci_
                    jc = j - nline_slots
                    g = goff[c] + j
                    gc = coff[c] + jc
                    a, bb = pp_coef(s)
                    cA[p, g] = a
                    cB[p, g] = bb
                    gg = gl[s]
                    ub = (gg["dx"] * xoff + gg["dy"] * y - gg["c0u"]) * gg["s"]
                    if jc >= ncap[c] - nabs[c]:
                        # abs flavor: At = |u - L/2| * SC ; E = max(At - h', 0)
                        rS[p, gc] = gg["dx"] * gg["s"] * SC
                        rB[p, gc] = (ub - gg["L"] / 2.0) * SC
                        hT[p, aoff[c] + (jc - (ncap[c] - nabs[c]))] = gg["L"] / 2.0 * SC
                    elif kind == 1:
                        rS[p, gc] = gg["dx"] * gg["s"] * SC
                        rB[p, gc] = (ub - gg["L"]) * SC
                    else:
                        rS[p, gc] = -gg["dx"] * gg["s"] * SC
                        rB[p, gc] = -ub * SC
        tb = np.zeros((P, L["TB"]))
        tb[:, L["cA"] : L["cA"] + G] = cA
        tb[:, L["cB"] : L["cB"] + G] = cB
        tc = np.zeros((P, L["TC"]))
        if NC:
            tc[:, L["rS"] : L["rS"] + NC] = rS
            tc[:, L["rB"] : L["rB"] + NC] = rB
        if NA:
            tc[:, L["hT"] : L["hT"] + NA] = hT
        if NN:
            tc[:, L["sA"] : L["sA"] + NN] = sA
            tc[:, L["sB"] : L["sB"] + NN] = sB
        tc[:, L["fin"]] = -1.0 / SC
        tc[:, L["fin"] + 1] = thr
        in_maps.append({"tb": tb.astype(np.float32), "tc": tc.astype(np.float32)})
    return in_maps


def kernel(**inputs):
    from concourse.bass_utils import run_bass_kernel_spmd

    images = np.asarray(inputs["images"])
    trajectories = np.asarray(inputs["trajectories"])
    line_width = inputs["line_width"]
    assert images.shape == (B, C, H, W), images.shape

    struct, assign, thr, geo = _plan(trajectories, line_width)
    progs = _state.setdefault("progs", {})
    if struct not in progs:
        progs[struct] = _build_program(struct)
    nc = progs[struct]

    in_maps = _prep_inputs(trajectories, struct, assign, thr, geo)
    res = run_bass_kernel_spmd(nc, in_maps, list(range(NCORE))).results
    ncomp = len(struct[0])
    out = np.zeros((B, H, W), np.float32)
    for core in range(NCORE):
        blk = res[core]["out"].astype(np.float32)  # [P, ncomp*QW]
        for c in range(ncomp):
            for p in range(P):
                ent = assign[core][c][p]
                if ent is None:
                    continue
                b, y, h = ent[0], ent[1], ent[2]
                out[b, y, h * QW : (h + 1) * QW] = blk[p, c * QW : (c + 1) * QW]
    full = np.broadcast_to(out[:, None, :, :], (B, C, H, W)).copy()
    return full


if __name__ == "__main__":
    rng = np.random.default_rng(0)
    ins = {
        "images": rng.standard_normal((B, C, H, W)).astype(np.float32),
        "trajectories": np.concatenate(
            [
                np.broadcast_to(np.linspace(0, 1, K, dtype=np.float32), (B, K))[..., None],
                rng.uniform(0, W - 1, (B, K, 2)).astype(np.float32),
                np.ones((B, K, 1), np.float32),
            ],
            axis=-1,
        ),
        "line_width": 3,
    }
    out = kernel(**ins)
    print(out.shape, out.dtype, out.min(), out.max())
